# revision 2
# baseline (speedup 1.0000x reference)
"""Chamfer distance loss kernel for Trainium2 (8 NeuronCores).

template [4, 8192, 3] f32, source [4, 8192, 3] f32 ->
scalar 0.5*(mean_n sqrt(min_m d2) + mean_m sqrt(min_n d2)).

Core c handles batch b = c//2, template half h = c%2.

Per-core pipeline (per 128-row n-tile, 32 tiles):
  e[n, m] = t.s - 0.5||t||^2 - 0.5||s||^2 = -0.5*d2, via K=13 fp16
  split-precision matmul. Matmuls cycle 4 PE row-groups (weights+moving
  operand replicated at partition bases 0/32/64/96) so up to 4 matmuls
  stream concurrently -> ~4x PE throughput at the cold 1.2 GHz clock.
  Scalar engine converts PSUM f32 -> SBUF f16 (4x 2048 per tile).
  ONE custom fused DVE op per tile (MAXCOLROW_ANT, runtime-registered
  with a hand-written 2x uop program):
      cm = max(cm, e)            (col-max accumulate, elementwise out)
      racc[:, ti] = rowmax(e)    (accumulator taps raw Src0)
  Row/col results reduced on host (tiny arrays).
"""

import copy

import numpy as np

F16 = np.float16
F32 = np.float32

B, N, M, D = 4, 8192, 8192, 3
N_CORES = 8
NSHARD = N // 2          # template rows per core (4096)
NT = NSHARD // 128       # n-tiles per core (32)
MG = M // 2048           # psum groups per n-tile (4)
K = 13                   # augmented contraction dim
KP = 109                 # replicated operand partitions (96 + 13)

_NC_CACHE = {}
_REGISTERED = {}


# --------------------------------------------------------------------------
# custom fused DVE op: out = max(in0, in1); accum_out = max-reduce(in0)
# --------------------------------------------------------------------------

def _maxcolrow_reference(in0, in1, s0, s1, imm2):
    out = np.maximum(in0.astype(np.float32), in1.astype(np.float32))
    P = out.shape[0]
    acc = in0.astype(np.float32).reshape(P, -1).max(axis=-1, keepdims=True)
    return out, np.maximum(acc, -3.402823466e38)


def _register_maxcolrow():
    if "op" in _REGISTERED:
        return _REGISTERED["op"]

    from concourse import dve_ops
    from concourse.dve_ops import DveOp
    from concourse.dve_spec import Spec, Src0, Src1, maxx, AluOp, lower
    from concourse.dve_uop import (
        DveOpSpec, InpSel, OutSel, OutPath, AluInp, DelayInp,
    )

    NAME = "MAXCOLROW_ANT"
    spec = Spec(body=maxx(Src0, Src1), accum=AluOp.MAX,
                reference=_maxcolrow_reference)

    op = DveOp.__new__(DveOp)
    object.__setattr__(op, "name", NAME)
    object.__setattr__(op, "spec", spec)
    object.__setattr__(op, "subdim", False)
    object.__setattr__(op, "uops_sha", {})
    object.__setattr__(op, "perf_en", {"v3": True})

    dve_ops.OPS.append(op)
    row = dve_ops._CUSTOM_DVE_ROW_BASE + len(dve_ops.OPS) - 1
    dve_ops._SUB_OPCODE_FOR_NAME[NAME] = row
    dve_ops.CUSTOM_DVE_SPECS[NAME] = spec

    uops_1x = lower(spec, ver="v3")
    assert len(uops_1x) == 2

    MAX = AluOp.MAX
    BYP = AluOp.BYPASS
    PA = AluInp.PREV_ALU_OUT
    CA = AluInp.CURR_ALU_OUT
    D0, D1, D2, D3, D4 = (AluInp.PREV_DELAY_0, AluInp.PREV_DELAY_1,
                          AluInp.PREV_DELAY_2, AluInp.PREV_DELAY_3,
                          AluInp.PREV_DELAY_4)
    dPA = DelayInp.PREV_ALU_OUT
    dPD = DelayInp.PREV_DELAY

    # 1x: retarget the accum stage (uop1 stage1) to raw Src0 (PREV_DELAY_0)
    u1x = uops_1x[1].datapath_config[1]
    assert u1x.op == MAX and u1x.alu_src0 == CA and u1x.alu_src1 == PA
    u1x.alu_src1 = D0

    u0, u1 = copy.deepcopy(uops_1x[0]), copy.deepcopy(uops_1x[1])
    for u in (u0, u1):
        u.inp[4] = InpSel.SRC_0_HI
        u.inp[5] = InpSel.SRC_1_HI
        u.inp_enable[4] = 1
        u.inp_enable[5] = 1

    def cfg(u, st, op_, a, b, delay, delay_en, out_a):
        d = u.datapath_config[st]
        d.op = op_
        d.alu_src0 = a
        d.alu_src1 = b
        d.delay = list(delay) + [dPD] * (7 - len(delay))
        d.delay_enable = list(delay_en) + [0] * (7 - len(delay_en))
        d.alu_out_enable = 1
        d.alu_out_a_enable = out_a

    # steady: stage0 body_lo=MAX(S0,S1); stage1 body_hi=MAX(S0H,S1H);
    # stage2 pairmax_e=MAX(S0,S0H); stage3 accum=MAX(CURR,pairmax);
    # body_lo/hi ride delay lanes 1/2 to the write ports.
    cfg(u1, 0, MAX, D0, D1, [dPD, dPD, dPD, dPD, dPD], [1, 0, 1, 1, 1], 0)
    cfg(u1, 1, MAX, D3, D4, [dPD, dPA, dPD, dPD, dPD], [1, 1, 0, 1, 0], 0)
    cfg(u1, 2, MAX, D0, D3, [dPD, dPD, dPA, dPD, dPD], [0, 1, 1, 0, 0], 0)
    cfg(u1, 3, MAX, CA, PA, [dPD, dPD, dPD, dPD, dPD], [0, 1, 1, 0, 0], 1)
    for st in range(4, 8):
        cfg(u1, st, BYP, PA, PA, [dPD, dPD, dPD, dPD, dPD],
            [0, 1, 1, 0, 0], 1)
    u1.out[OutPath.WR0_LO] = OutSel.DELAY_1
    u1.out[OutPath.WR0_HI] = OutSel.DELAY_2
    u1.out_enable[OutPath.WR0_LO] = 1
    u1.out_enable[OutPath.WR0_HI] = 1
    u1.out_enable[OutPath.WR1_LO] = 0
    u1.out_enable[OutPath.WR1_HI] = 0

    # seed: carry MAX_NEG on lane2 to stage3, seed the accumulator flop
    cfg(u0, 0, MAX, D0, D1, [dPD, dPD, dPD, dPD, dPD], [1, 0, 1, 1, 1], 0)
    cfg(u0, 1, MAX, D3, D4, [dPD, dPA, dPD, dPD, dPD], [1, 1, 1, 1, 0], 0)
    cfg(u0, 2, MAX, D0, D3, [dPD, dPD, dPD, dPD, dPD], [0, 1, 1, 0, 0], 0)
    cfg(u0, 3, BYP, D2, D2, [dPD, dPD, dPD, dPD, dPD], [0, 1, 1, 0, 0], 1)
    for st in range(4, 8):
        cfg(u0, st, BYP, PA, PA, [dPD, dPD, dPD, dPD, dPD],
            [0, 1, 1, 0, 0], 1)
    for p in (OutPath.WR0_LO, OutPath.WR0_HI, OutPath.WR1_LO, OutPath.WR1_HI):
        u0.out_enable[p] = 0

    compiled = DveOpSpec(
        name=NAME, opcode=row, uops=uops_1x, uops_2x=[u0, u1],
        perf_max=1, rd1_en=True,
    )
    compiled.validate("v3")
    dve_ops._COMPILE_CACHE[(NAME, "v3")] = compiled
    _REGISTERED["op"] = op
    return op


def _emit_maxcolrow(nc, out, in0, in1, accum_out):
    op = _register_maxcolrow()
    inst = nc.vector._custom_dve(
        op, out=out, in0=in0, in1=in1, accum_out=accum_out)
    target = getattr(inst, "instr", inst)
    try:
        target.perf_max = 1
    except AttributeError:
        inst.perf_max = 1
    return inst


# --------------------------------------------------------------------------
# device program
# --------------------------------------------------------------------------

def _build_nc():
    import concourse.bacc as bacc
    import concourse.mybir as mybir
    from concourse.tile import TileContext

    f16 = mybir.dt.float16
    f32 = mybir.dt.float32

    nc = bacc.Bacc()
    lhsT = nc.declare_dram_parameter("lhsT", [KP, NSHARD], f16, isOutput=False)
    rhs = nc.declare_dram_parameter("rhs", [KP, M], f16, isOutput=False)
    rowmax_o = nc.declare_dram_parameter("rowmax", [128, NT], f32,
                                         isOutput=True)
    colmax_a_o = nc.declare_dram_parameter("colmaxA", [128, M], f16,
                                           isOutput=True)
    colmax_b_o = nc.declare_dram_parameter("colmaxB", [128, M], f16,
                                           isOutput=True)

    with TileContext(nc) as tc:
        with (
            tc.tile_pool(name="const", bufs=1) as cpool,
            tc.tile_pool(name="psum", bufs=2, space="PSUM") as ppool,
            tc.tile_pool(name="ebuf", bufs=2) as epool,
        ):
            lhsT_sb = cpool.tile([KP, NSHARD], f16)
            nc.gpsimd.dma_start(lhsT_sb[:], lhsT[:])
            rhs_q = []
            for q in range(MG):
                t = cpool.tile([KP, M // MG], f16, tag=f"rhsq{q}")
                nc.gpsimd.dma_start(
                    t[:], rhs[:, q * (M // MG):(q + 1) * (M // MG)])
                rhs_q.append(t)

            cmaxA = cpool.tile([128, M], f16)
            cmaxB = cpool.tile([128, M], f16)
            racc = cpool.tile([128, NT], f32)
            nc.gpsimd.memset(cmaxA[:], -60000.0)
            nc.gpsimd.memset(cmaxB[:], -60000.0)

            for ti in range(NT):
                e = epool.tile([128, M], f16, tag="e")
                lsl = slice(ti * 128, (ti + 1) * 128)
                for g in range(MG):
                    ps = ppool.tile([128, 2048], f32, tag="ps")
                    for j in range(4):
                        q = (g * 4 + j) % 4
                        nc.tensor.matmul(
                            ps[:, j * 512:(j + 1) * 512],
                            lhsT_sb[32 * q:32 * q + K, lsl],
                            rhs_q[g][32 * q:32 * q + K,
                                     j * 512:(j + 1) * 512],
                            start=True,
                            stop=True,
                            tile_position=(32 * q, 0),
                        )
                    nc.scalar.copy(e[:, g * 2048:(g + 1) * 2048], ps[:])
                cm = cmaxA if ti < NT // 2 else cmaxB
                _emit_maxcolrow(nc, out=cm[:], in0=e[:], in1=cm[:],
                                accum_out=racc[:, ti:ti + 1])
                if ti == NT // 2 - 1:
                    nc.gpsimd.dma_start(colmax_a_o[:], cmaxA[:])

            nc.gpsimd.dma_start(rowmax_o[:], racc[:])
            nc.gpsimd.dma_start(colmax_b_o[:], cmaxB[:])
    return nc


def get_nc():
    if "nc" not in _NC_CACHE:
        nc = _build_nc()
        nc.finalize()
        _NC_CACHE["nc"] = nc
    return _NC_CACHE["nc"]


def _split16(x32):
    hi = x32.astype(F16)
    lo = (x32 - hi.astype(F32)).astype(F16)
    return hi, lo


def _build_lhsT(t):
    """t: [n, 3] f32 -> [109, n] f16 stationary operand, 4x replicated."""
    n = t.shape[0]
    th, tl = _split16(t)
    t2 = (t * t).sum(axis=1, dtype=F32)
    u = -0.5 * t2
    uh, ul = _split16(u)
    blk = np.empty((K, n), dtype=F16)
    blk[0:3] = th.T
    blk[3:6] = tl.T
    blk[6:9] = th.T
    blk[9] = uh
    blk[10] = ul
    blk[11] = 1.0
    blk[12] = 1.0
    out = np.zeros((KP, n), dtype=F16)
    for q in range(4):
        out[32 * q:32 * q + K] = blk
    return out


def _build_rhs(s):
    """s: [m, 3] f32 -> [109, m] f16 moving operand, 4x replicated."""
    m = s.shape[0]
    sh, sl = _split16(s)
    s2 = (s * s).sum(axis=1, dtype=F32)
    v = -0.5 * s2
    vh, vl = _split16(v)
    blk = np.empty((K, m), dtype=F16)
    blk[0:3] = sh.T
    blk[3:6] = sh.T
    blk[6:9] = sl.T
    blk[9] = 1.0
    blk[10] = 1.0
    blk[11] = vh
    blk[12] = vl
    out = np.zeros((KP, m), dtype=F16)
    for q in range(4):
        out[32 * q:32 * q + K] = blk
    return out


def make_in_maps(template, source):
    template = np.asarray(template, dtype=F32)
    source = np.asarray(source, dtype=F32)
    in_maps = []
    for c in range(N_CORES):
        b, h = divmod(c, 2)
        t = template[b, h * NSHARD:(h + 1) * NSHARD]
        s = source[b]
        in_maps.append({"lhsT": _build_lhsT(t), "rhs": _build_rhs(s)})
    return in_maps


def finalize(results):
    row_sqrts = []
    for c in range(N_CORES):
        rm = np.asarray(results[c]["rowmax"], dtype=F32)
        row_sqrts.append(np.sqrt(np.maximum(-2.0 * rm, 0.0), dtype=F32).ravel())
    c01 = np.mean(np.concatenate(row_sqrts), dtype=F32)

    col_sqrts = []
    for b in range(B):
        cm = np.maximum(
            np.maximum(np.asarray(results[2 * b]["colmaxA"]),
                       np.asarray(results[2 * b]["colmaxB"])),
            np.maximum(np.asarray(results[2 * b + 1]["colmaxA"]),
                       np.asarray(results[2 * b + 1]["colmaxB"])),
        ).max(axis=0).astype(F32)
        col_sqrts.append(np.sqrt(np.maximum(-2.0 * cm, 0.0), dtype=F32))
    c10 = np.mean(np.concatenate(col_sqrts), dtype=F32)
    return np.float32((c01 + c10) * 0.5)


def kernel(template, source):
    from concourse.bass_utils import run_bass_kernel_spmd

    nc = get_nc()
    in_maps = make_in_maps(template, source)
    res = run_bass_kernel_spmd(nc, in_maps, list(range(N_CORES))).results
    return finalize(res)
